# revision 44
# baseline (speedup 1.0000x reference)
"""Trainium2 Bass kernel for nn_DifferentiableKalmanFilter.

Strategy
--------
The 4x4 covariance recursion is batch-independent and, by x/y symmetry,
collapses to two scalar gain sequences k_p(t), k_v(t) computed on the host.
Per batch row the filter is a 2-state linear recurrence per coordinate:

    s_t = s_{t-1} @ M_t + z_t * g_t,   s = [p, v],
    M_t = [[1, 0], [dt - k_p(t), 1 - k_v(t)]],  g_t = [k_p(t), k_v(t)]

Unrolling 252 steps per chunk turns the recurrence into matmuls with
host-precomputed weights, TIME-MAJOR: stationary W [K, out-cols], moving
operand = "stacks" [K, batch] whose rows are [p_carry, v_carry, z...];
PSUM output rows are time steps.  The contraction is split into two
accumulating K-tiles (K=128: carries + z_0..125, and K=126: z_126..251),
and the 252 outputs into two row-blocks of <=128:

    A: pos_0..125 / vel_0..125          (K-tile 1 only)
    B: [pos_251, vel_251, pos_126..250] / vel_126..250   (both K-tiles)

The chunk-end state [pos_251, vel_251] lands in B's PSUM partitions 0:2;
one partition-aligned engine copy (f32->fp16) moves it straight into the
next chunk's stack rows 0:2.  With only 4 chain steps and ~5.7us of
store traffic per chunk, the serial chain hides completely under the
DMA-bound steady state.  A few junk warm-up matmuls at kernel start
ramp the PE p-state so the first real chunk runs at full clock.

The rel-err budget (2e-2) is huge, so a single fp16 x fp16 product with
f32 PSUM accumulation suffices (~1e-3), and outputs are stored as fp16
(halving store traffic); the host converts back to f32.  All PSUM->SBUF
copies run on ACT and DVE only (GPSIMD cannot access PSUM).

Sharding: pure data parallel over batch across 8 cores (1024 rows/core).
Device output is time-major [T, 4, BC] fp16 (pos_x, pos_y, vel_x, vel_y
planes); the host transposes/interleaves at the end.
"""

import numpy as np

import concourse.bass as bass
import concourse.tile as tile
from concourse import bacc, mybir
from concourse.bass_utils import run_bass_kernel_spmd

# Problem shape (hardcoded per harness contract)
B = 8192
T = 1024
NCORES = 8
BC = B // NCORES  # 1024 batch rows per core
L = 252  # full-chunk length (two K-tiles of 126)
H = 126  # K-tile z rows / A-block length
CH = [(c * L, L) for c in range(T // L)] + (
    [(T - T % L, T % L)] if T % L else []
)  # [(t0, Lc)]: 4 x 252 + 16
NCH = len(CH)
GH = BC // 2  # matmul moving-operand half (PSUM bank = 512 f32)

# engine per plane-copy: (pos-A, pos-B, vel-A, vel-B) x (x, y); A=ACT, D=DVE
DEFAULT_CFG = {
    "pAx": "A", "pBx": "A", "vAx": "D", "vBx": "D",
    "pAy": "A", "pBy": "D", "vAy": "D", "vBy": "A",
    "ps": 4, "ot": 3,
    "k2first": False,  # B blocks: run the carry-independent K-tile-2 mm first
    "st1split": False,  # split the A-block store into pos/vel plane pairs
    "carry_dma": False,  # carry via SWDGE DMA from o_t instead of engine copy
    "nwarm": 8,  # PE p-state warm-up matmuls
}


# ---------------------------------------------------------------- host math
def _gains(dt, q_pos, q_vel, r_vel):
    """Scalar Kalman gain sequences in float64 (exact vs fp32 reference)."""
    dt = float(dt)
    r_reg = float(np.float32(r_vel) + np.float32(1e-6))
    q_pos = float(q_pos)
    q_vel = float(q_vel)
    a, b, c = 1.0, 0.0, 1.0  # P blocks [[a, b], [b, c]] per coordinate
    kp = np.zeros(T)
    kv = np.zeros(T)
    for t in range(T):
        ap = a + 2 * dt * b + dt * dt * c + q_pos
        bp = b + dt * c
        cp = c + q_vel
        den = cp + r_reg
        kp[t] = bp / den
        kv[t] = cp / den
        a = ap - kp[t] * bp
        b = bp * r_reg / den
        c = cp * r_reg / den
    return kp, kv


def _build_weights(kp, kv, dt):
    """Per-chunk stationary weights, fp16.

    Full chunks: Wk1 [128, 504] (rows [p, v, z_0..z_125]; col blocks
    posA(126) | posB(127) | velA(126) | velB(125)) and Wk2 [126, 252]
    (rows z_126..z_251; cols posB | velB).  posB = [pos_251, vel_251,
    pos_126..pos_250].  Last chunk: [Lc+2, 2*Lc] with col order
    [pos_last, vel_last, pos_0.. | vel_0..].

    Returns (wk1 [128, nuf, 504], wk2 [128, nuf, 252], w_last, chunk_map).
    """
    dt = float(dt)
    pairs = []
    w_last = None
    for t0, Lc in CH:
        U = np.zeros((Lc, 2))
        C = np.eye(2)
        P = np.zeros((Lc + 2, Lc))
        V = np.zeros((Lc + 2, Lc))
        for u in range(Lc):
            t = t0 + u
            M = np.array([[1.0, 0.0], [dt - kp[t], 1.0 - kv[t]]])
            U[:u] = U[:u] @ M
            U[u] = (kp[t], kv[t])
            C = C @ M
            P[0, u], V[0, u] = C[0, 0], C[0, 1]
            P[1, u], V[1, u] = C[1, 0], C[1, 1]
            P[2 : 2 + u + 1, u] = U[: u + 1, 0]
            V[2 : 2 + u + 1, u] = U[: u + 1, 1]
        if Lc == L:
            assert np.abs(P[128:, 0:H]).max() == 0 and np.abs(V[128:, 0:H]).max() == 0
            posB = np.concatenate(
                [P[:, L - 1 : L], V[:, L - 1 : L], P[:, H : L - 1]], axis=1
            )
            velB = V[:, H : L - 1]
            W = np.concatenate([P[:, 0:H], posB, V[:, 0:H], velB], axis=1)
            pairs.append(
                (W[0:128].astype(np.float16),
                 np.concatenate([posB, velB], axis=1)[128:].astype(np.float16))
            )
        else:
            W = np.zeros((Lc + 2, 2 * Lc))
            W[:, 0] = P[:, Lc - 1]
            W[:, 1] = V[:, Lc - 1]
            W[:, 2 : Lc + 1] = P[:, : Lc - 1]
            W[:, Lc + 1 :] = V[:, : Lc - 1]
            w_last = W.astype(np.float16)

    # dedupe full chunks (gains converge -> steady chunks share weights)
    chunk_map = []
    uniq = []
    for c, (w1, w2) in enumerate(pairs):
        found = None
        for ui, u in enumerate(uniq):
            if np.array_equal(w1, pairs[u][0]) and np.array_equal(w2, pairs[u][1]):
                found = ui
                break
        if found is None:
            uniq.append(c)
            found = len(uniq) - 1
        chunk_map.append(found)

    # pre-transposed to the SBUF layout so each load is one contiguous DMA
    wk1 = np.zeros((128, len(uniq), 2 * L), dtype=np.float16)
    wk2 = np.zeros((128, len(uniq), L), dtype=np.float16)
    for ui, u in enumerate(uniq):
        wk1[:, ui, :] = pairs[u][0]
        wk2[0:H, ui, :] = pairs[u][1]
    return wk1, wk2, w_last, chunk_map


# ---------------------------------------------------------------- bass build
def _build_nc(nuf, chunk_map, has_last, cfg=None):
    cfg = {**DEFAULT_CFG, **(cfg or {})}
    f32 = mybir.dt.float32
    f16 = mybir.dt.float16

    nc = bacc.Bacc(
        "TRN2",
        target_bir_lowering=False,
        debug=False,
        enable_asserts=False,
    )
    zs_d = nc.dram_tensor("zs", [NCH, 2, 128, 2, BC], f16, kind="ExternalInput").ap()
    wk1_d = nc.dram_tensor("wk1", [128, nuf, 2 * L], f16, kind="ExternalInput").ap()
    wk2_d = nc.dram_tensor("wk2", [128, nuf, L], f16, kind="ExternalInput").ap()
    if has_last:
        _, Ll = CH[-1]
        wl_d = nc.dram_tensor("wlast", [Ll + 2, 2 * Ll], f16, kind="ExternalInput").ap()
    out_d = nc.dram_tensor("out", [T, 4, BC], f16, kind="ExternalOutput").ap()

    with tile.TileContext(nc) as tc:
        with (
            tc.tile_pool(name="wpool", bufs=1) as wpool,
            tc.tile_pool(name="stacks", bufs=1) as spool,
            tc.tile_pool(name="outp", bufs=cfg["ot"]) as opool,
            tc.tile_pool(name="psum", bufs=cfg["ps"], space="PSUM") as pspool,
        ):
            # stacks: per full chunk two K-tiles; S1 rows [p_carry, v_carry,
            # z_0..z_125], S2 rows [z_126..z_251].  Chunk 0's carry rows
            # (p0, 0) come baked from the host; later chunks get theirs via
            # a tiny SBUF->SBUF DMA from the previous chunk's out tile.
            # PE warm-up: the cost model's p-state ramp needs ~3us of tensor
            # activity before the PE reaches full clock.  A dozen junk
            # matmuls on a memset tile during the load phase ramp it up so
            # the first real chunk runs at full speed.
            jt = wpool.tile([128, 512], f16)
            nc.vector.memset(jt[:], 0.0)
            wps = pspool.tile([128, BC], f32, tag="ps", name="warm")
            NWARM = cfg["nwarm"]
            for i in range(NWARM):
                nc.tensor.matmul(
                    wps[0:2, 0:GH], jt[:, 0:2], jt[:, 0:GH],
                    start=(i == 0), stop=(i == NWARM - 1),
                )

            s1s, s2s = [], []
            for c, (t0, Lc) in enumerate(CH):
                s1s.append(
                    spool.tile([128, 2, BC], f16, tag=f"s1_{c}", name=f"s1_{c}")
                )
                s2s.append(
                    spool.tile([128, 2, BC], f16, tag=f"s2_{c}", name=f"s2_{c}")
                    if Lc == L
                    else None
                )
            nc.sync.dma_start(s1s[0][:], zs_d[0, 0])
            w1_t = wpool.tile([128, nuf, 2 * L], f16)
            nc.sync.dma_start(w1_t[:], wk1_d)
            w2_t = wpool.tile([128, nuf, L], f16)
            nc.sync.dma_start(w2_t[:], wk2_d)
            if has_last:
                _, Ll = CH[-1]
                wl_t = wpool.tile([Ll + 2, 2 * Ll], f16)
                nc.sync.dma_start(wl_t[:], wl_d)
            nc.sync.dma_start(s2s[0][0:H], zs_d[0, 1, 0:H])
            for c, (t0, Lc) in enumerate(CH[1:], start=1):
                if Lc == L:
                    nc.sync.dma_start(s1s[c][2:128], zs_d[c, 0, 2:128])
                    nc.sync.dma_start(s2s[c][0:H], zs_d[c, 1, 0:H])
                else:
                    nc.sync.dma_start(s1s[c][2 : 2 + Lc], zs_d[c, 0, 2 : 2 + Lc])

            engs = {"A": nc.scalar, "D": nc.vector}

            def cpy(ek, dst, src):
                """Copy dst<-src; ek "A"/"D", or a pair = 512-wide halves."""
                parts = (
                    [(ek, slice(0, BC))]
                    if isinstance(ek, str)
                    else [(ek[0], slice(0, GH)), (ek[1], slice(GH, BC))]
                )
                for e, bsl in parts:
                    if e == "A":
                        nc.scalar.mul(dst[..., bsl], src[..., bsl], 1.0)
                    else:
                        nc.vector.tensor_copy(dst[..., bsl], src[..., bsl])

            def full_chunk(c):
                t0, _ = CH[c]
                ci = chunk_map[c]
                o_t = opool.tile([128, 2, 4, BC], f16, tag="out")
                # B blocks first (they feed the serial carry chain), and
                # within each accumulation the K-tile-2 matmul (pure z, no
                # carry dependency) runs FIRST so the PE has work while the
                # previous chunk's carry hop is still in flight.
                for cd, x in ((0, "x"), (1, "y")):
                    pB = pspool.tile([128, BC], f32, tag="ps", name=f"pB{c}{x}")
                    for g in range(2):
                        gsl = slice(g * GH, (g + 1) * GH)
                        mm_k1 = (
                            pB[0 : H + 1, gsl], w1_t[0:128, ci, H : 2 * H + 1],
                            s1s[c][0:128, cd, gsl],
                        )
                        mm_k2 = (
                            pB[0 : H + 1, gsl], w2_t[0:H, ci, 0 : H + 1],
                            s2s[c][0:H, cd, gsl],
                        )
                        first, second = (
                            (mm_k2, mm_k1) if cfg["k2first"] else (mm_k1, mm_k2)
                        )
                        nc.tensor.matmul(*first, start=True, stop=False)
                        nc.tensor.matmul(*second, start=False, stop=True)
                    if c + 1 < NCH and not cfg["carry_dma"]:
                        # carry chain: chunk-end [p, v] -> next stack rows 0:2
                        cpy(cfg["pB" + x], s1s[c + 1][0:2, cd], pB[0:2, :])
                    cpy(cfg["pB" + x], o_t[0 : H + 1, 1, cd], pB[0 : H + 1, :])
                    if c + 1 < NCH and cfg["carry_dma"]:
                        # carry rides the B out-copy: [p, v] are already fp16
                        # in o_t rows 0:2; a tiny SWDGE DMA (idle Pool queue)
                        # moves them into the next chunk's stack
                        nc.gpsimd.dma_start(s1s[c + 1][0:2, cd], o_t[0:2, 1, cd])
                for cd, x in ((0, "x"), (1, "y")):
                    # A blocks before vB: the big A store (all 4 planes, 1MB)
                    # only needs pA+vA, so it becomes ready earlier
                    pA = pspool.tile([128, BC], f32, tag="ps", name=f"pA{c}{x}")
                    for g in range(2):
                        gsl = slice(g * GH, (g + 1) * GH)
                        nc.tensor.matmul(
                            pA[0:H, gsl], w1_t[0:128, ci, 0:H],
                            s1s[c][0:128, cd, gsl], start=True, stop=True,
                        )
                    cpy(cfg["pA" + x], o_t[0:H, 0, cd], pA[0:H, :])
                    vA = pspool.tile([128, BC], f32, tag="ps", name=f"vA{c}{x}")
                    for g in range(2):
                        gsl = slice(g * GH, (g + 1) * GH)
                        nc.tensor.matmul(
                            vA[0:H, gsl], w1_t[0:128, ci, 2 * H + 1 : 3 * H + 1],
                            s1s[c][0:128, cd, gsl], start=True, stop=True,
                        )
                    cpy(cfg["vA" + x], o_t[0:H, 0, 2 + cd], vA[0:H, :])
                for cd, x in ((0, "x"), (1, "y")):
                    vB = pspool.tile([128, BC], f32, tag="ps", name=f"vB{c}{x}")
                    for g in range(2):
                        gsl = slice(g * GH, (g + 1) * GH)
                        nc.tensor.matmul(
                            vB[0 : H - 1, gsl], w1_t[0:128, ci, 3 * H + 1 : 2 * L],
                            s1s[c][0:128, cd, gsl], start=True, stop=False,
                        )
                        nc.tensor.matmul(
                            vB[0 : H - 1, gsl], w2_t[0:H, ci, H + 1 : L],
                            s2s[c][0:H, cd, gsl], start=False, stop=True,
                        )
                    cpy(cfg["vB" + x], o_t[0 : H - 1, 1, 2 + cd], vB[0 : H - 1, :])
                # stores: A block rows map to t0..t0+125 directly; B block is
                # rotated (row 0 = pos_251, row 1 = vel_251, rows 2: = pos_126..)
                tm = t0 + H
                tl = t0 + L - 1
                if cfg["st1split"]:
                    nc.sync.dma_start(out_d[t0:tm, 0:2], o_t[0:H, 0, 0:2])
                    nc.sync.dma_start(out_d[t0:tm, 2:4], o_t[0:H, 0, 2:4])
                else:
                    nc.sync.dma_start(out_d[t0:tm, 0:4], o_t[0:H, 0, 0:4])
                nc.sync.dma_start(out_d[tm:tl, 0:2], o_t[2 : H + 1, 1, 0:2])
                nc.sync.dma_start(out_d[tm:tl, 2:4], o_t[0 : H - 1, 1, 2:4])
                nc.sync.dma_start(
                    out_d[tl : tl + 1].rearrange("t (r p) b -> (t r) p b", r=2),
                    o_t[0:2, 1, 0:2],
                )

            def last_chunk(c):
                t0, Lc = CH[c]
                K = Lc + 2
                o_t = opool.tile([128, 2, 4, BC], f16, tag="out")
                for cd, x in ((0, "x"), (1, "y")):
                    pp = pspool.tile([128, BC], f32, tag="ps")
                    for g in range(2):
                        gsl = slice(g * GH, (g + 1) * GH)
                        nc.tensor.matmul(
                            pp[0 : Lc + 1, gsl], wl_t[:, 0 : Lc + 1],
                            s1s[c][0:K, cd, gsl], start=True, stop=True,
                        )
                    cpy(cfg["pB" + x], o_t[0 : Lc + 1, 0, cd], pp[0 : Lc + 1, :])
                    vv = pspool.tile([128, BC], f32, tag="ps")
                    for g in range(2):
                        gsl = slice(g * GH, (g + 1) * GH)
                        nc.tensor.matmul(
                            vv[0 : Lc - 1, gsl], wl_t[:, Lc + 1 : 2 * Lc],
                            s1s[c][0:K, cd, gsl], start=True, stop=True,
                        )
                    cpy(cfg["vB" + x], o_t[0 : Lc - 1, 0, 2 + cd], vv[0 : Lc - 1, :])
                tl = t0 + Lc - 1
                nc.sync.dma_start(out_d[t0:tl, 0:2], o_t[2 : Lc + 1, 0, 0:2])
                nc.sync.dma_start(out_d[t0:tl, 2:4], o_t[0 : Lc - 1, 0, 2:4])
                nc.sync.dma_start(
                    out_d[tl : tl + 1].rearrange("t (r p) b -> (t r) p b", r=2),
                    o_t[0:2, 0, 0:2],
                )

            for c, (t0, Lc) in enumerate(CH):
                if Lc == L:
                    full_chunk(c)
                else:
                    last_chunk(c)
    nc.compile()
    return nc


# ---------------------------------------------------------------- entry
def _prepare(pred_vel, dt, p0, q_pos, q_vel, r_vel):
    kp, kv = _gains(dt, q_pos, q_vel, r_vel)
    wk1, wk2, w_last, chunk_map = _build_weights(kp, kv, dt)
    nuf = wk1.shape[1]

    pred_vel = np.asarray(pred_vel, dtype=np.float32)
    p0 = np.asarray(p0, dtype=np.float32)
    in_maps = []
    for i in range(NCORES):
        pv = pred_vel[i * BC : (i + 1) * BC]  # (BC, T, 2)
        zt = np.ascontiguousarray(pv.transpose(1, 2, 0)).astype(np.float16)
        zs = np.zeros((NCH, 2, 128, 2, BC), dtype=np.float16)
        for c, (t0, Lc) in enumerate(CH):
            if Lc == L:
                zs[c, 0, 2:128] = zt[t0 : t0 + H]
                zs[c, 1, 0:H] = zt[t0 + H : t0 + L]
            else:
                zs[c, 0, 2 : 2 + Lc] = zt[t0 : t0 + Lc]
        zs[0, 0, 0] = p0[i * BC : (i + 1) * BC].T.astype(np.float16)  # p carry
        m = {"zs": zs, "wk1": wk1, "wk2": wk2}
        if w_last is not None:
            m["wlast"] = w_last
        in_maps.append(m)
    return nuf, chunk_map, w_last is not None, in_maps


def run(pred_vel, dt, p0, q_pos, q_vel, r_vel, trace=False, cfg=None, **spmd_kwargs):
    nuf, chunk_map, has_last, in_maps = _prepare(
        pred_vel, dt, p0, q_pos, q_vel, r_vel
    )
    nc = _build_nc(nuf, chunk_map, has_last, cfg=cfg)
    res = run_bass_kernel_spmd(
        nc, in_maps, core_ids=list(range(NCORES)), trace=trace, **spmd_kwargs
    )
    pos = np.empty((B, T, 2), dtype=np.float32)
    vel = np.empty((B, T, 2), dtype=np.float32)
    for i in range(NCORES):
        o = np.asarray(res.results[i]["out"], dtype=np.float32)  # (T, 4, BC)
        sl = slice(i * BC, (i + 1) * BC)
        pos[sl, :, 0] = o[:, 0].T
        pos[sl, :, 1] = o[:, 1].T
        vel[sl, :, 0] = o[:, 2].T
        vel[sl, :, 1] = o[:, 3].T
    return (pos, vel), res


def kernel(pred_vel, dt, p0, q_pos, q_vel, r_vel):
    (pos, vel), _ = run(pred_vel, dt, p0, q_pos, q_vel, r_vel, trace=False)
    return pos, vel
